# revision 5
# baseline (speedup 1.0000x reference)
"""Trainium2 Bass kernel for nn_Decoder (attention-LSTM caption decoder).

Strategy (8 NeuronCores):
  - Init matmuls (h0/c0 = flat @ W_ih/W_ic, C0 = rel @ W_iC) are K-sharded:
    each core reads 1/8 of the big weights (bf16), computes partial sums,
    and a ReduceScatter(add) hands core c the rows for its 4 batch elements.
  - The 19-step recurrent loop is batch-sharded (4 sequences per core) with
    all loop weights replicated in SBUF; no per-step collectives.
  - pred = h @ W_fc is hoisted out of the loop: h^T for all steps is stacked
    and one big matmul produces the predictions at the end.
  - Host (numpy) does: sort-by-length, embedding gather, layout transposes,
    bf16 casts, bias b_fc add + length masking of outputs.

All device tensors are laid out host-side as [128 partitions, ...] so every
DMA is a contiguous-line transfer.
"""

import numpy as np
import ml_dtypes

B, T = 32, 20
H, E, A, V = 256, 512, 256, 10000
P, D = 1024, 256
IN_FLAT = P * D          # 262144
REL = 128 * 128          # 16384
Tm = T - 1               # 19
NC_ = 8                  # cores
BL = B // NC_            # 4 sequences per core
KSH = IN_FLAT // NC_     # 32768 contraction rows per core
KSHC = KSH // 128        # 256 k-chunks per core
RSH = REL // NC_         # 2048
RSHC = RSH // 128        # 16

BF16 = ml_dtypes.bfloat16
F32 = np.float32

_CACHE = {}


def _build_nc():
    """Build + compile the SPMD Bass module (same NEFF for all 8 cores)."""
    import concourse.bacc as bacc
    import concourse.tile as tile
    from concourse import mybir

    dt = mybir.dt
    AF = mybir.ActivationFunctionType
    OP = mybir.AluOpType

    nc = bacc.Bacc("TRN2", target_bir_lowering=False, debug=False,
                   enable_asserts=False, num_devices=NC_)

    def din(name, shape, d=dt.bfloat16):
        return nc.dram_tensor(name, list(shape), d, kind="ExternalInput").ap()

    # per-core sharded inputs
    flatT = din("flatT", [128, KSHC, BL * NC_])          # rhs for init (N=32)
    Wih = din("Wih", [128, KSHC, H])
    Wic = din("Wic", [128, KSHC, H])
    relT = din("relT", [128, RSHC, BL * NC_])
    WiC = din("WiC", [128, RSHC, H])
    infoT = din("infoT", [128, 2, BL * P])               # [p, d-chunk, (b,pix)]
    infoc = din("infoc", [128, BL * 8, D])               # [pix, (b,po), d]
    wordsT = din("wordsT", [128, 4, Tm, BL])             # [p, e-chunk, t, b]
    # replicated weights
    Wenc = din("Wenc", [128, 2, A])
    Wdec = din("Wdec", [128, 2, A])
    Wfull = din("Wfull", [128, 2])
    Wg = din("Wg", [128, 8, 5 * H])
    Wmlp = din("Wmlp", [128, 4, H])
    Wfc = din("Wfc", [128, 2, V])
    benc = din("benc", [128, 2], dt.float32)
    bdec = din("bdec", [128, 2], dt.float32)
    bg = din("bg", [128, 10], dt.float32)
    bmlp = din("bmlp", [128, 2], dt.float32)
    bih = din("bih", [128, 2], dt.float32)
    bic = din("bic", [128, 2], dt.float32)
    biC = din("biC", [128, 2], dt.float32)
    ident4 = din("ident4", [BL, BL], dt.float32)
    ones_a = din("ones_a", [128, 1], dt.float32)
    ones_b = din("ones_b", [1, 128], dt.float32)

    preds_o = nc.dram_tensor("preds_o", [BL * Tm, V], dt.float32,
                             kind="ExternalOutput").ap()
    alph_o = nc.dram_tensor("alph_o", [Tm, 128, BL, 8], dt.float32,
                            kind="ExternalOutput").ap()

    cc_in = nc.dram_tensor("cc_in", [B, 3 * H], dt.float32).ap()
    cc_out = nc.dram_tensor("cc_out", [BL, 3, 2, 128], dt.float32).ap()

    with tile.TileContext(nc) as tc:
        from contextlib import ExitStack
        with ExitStack() as ctx:
            consts = ctx.enter_context(tc.tile_pool(name="consts", bufs=1))
            wstream = ctx.enter_context(tc.tile_pool(name="wstream", bufs=3))
            rpool = ctx.enter_context(tc.tile_pool(name="rpool", bufs=10))
            small = ctx.enter_context(tc.tile_pool(name="small", bufs=3))
            work = ctx.enter_context(tc.tile_pool(name="work", bufs=3))
            ps = ctx.enter_context(tc.tile_pool(name="ps", bufs=7, space="PSUM"))

            def cload(ap_dram, shape, d=dt.bfloat16, name=None):
                t = consts.tile(shape, d, tag=name)
                nc.sync.dma_start(out=t[:], in_=ap_dram)
                return t

            flatT_sb = cload(flatT, [128, KSHC, 32], name="flatT")
            relT_sb = cload(relT, [128, RSHC, 32], name="relT")
            WiC_sb = cload(WiC, [128, RSHC, H], name="WiC")
            infoT_sb = cload(infoT, [128, 2, BL * P], name="infoT")
            infoc_sb = cload(infoc, [128, BL * 8, D], name="infoc")
            wordsT_sb = cload(wordsT, [128, 4, Tm, BL], name="wordsT")
            Wenc_sb = cload(Wenc, [128, 2, A], name="Wenc")
            Wdec_sb = cload(Wdec, [128, 2, A], name="Wdec")
            Wfull_sb = cload(Wfull, [128, 2], name="Wfull")
            Wg_sb = cload(Wg, [128, 8, 5 * H], name="Wg")
            Wmlp_sb = cload(Wmlp, [128, 4, H], name="Wmlp")
            Wfc_sb = cload(Wfc, [128, 2, V], name="Wfc")
            benc_sb = cload(benc, [128, 2], dt.float32, "benc")
            bdec_sb = cload(bdec, [128, 2], dt.float32, "bdec")
            bg_sb = cload(bg, [128, 10], dt.float32, "bg")
            bmlp_sb = cload(bmlp, [128, 2], dt.float32, "bmlp")
            bih_sb = cload(bih, [128, 2], dt.float32, "bih")
            bic_sb = cload(bic, [128, 2], dt.float32, "bic")
            biC_sb = cload(biC, [128, 2], dt.float32, "biC")
            id4_sb = cload(ident4, [BL, BL], dt.float32, "id4")
            onesa_sb = cload(ones_a, [128, 1], dt.float32, "onesa")
            onesb_sb = cload(ones_b, [1, 128], dt.float32, "onesb")

            att1T_sb = consts.tile([128, 2, BL * P], dt.bfloat16, tag="att1T")
            hall_bf = consts.tile([128, 2, T, BL], dt.bfloat16, tag="hall")
            cC = consts.tile([128, 2, 2, BL], dt.float32, tag="cC")

            MM = nc.tensor.matmul

            # ---- init partial matmuls (K-sharded; out rows = global batch) ----
            ps_h0 = ps.tile([32, H], dt.float32, tag="bank", name="ps_h0")
            ps_c0 = ps.tile([32, H], dt.float32, tag="bank", name="ps_c0")
            ps_C0 = ps.tile([32, H], dt.float32, tag="bank", name="ps_C0")
            G = 8
            for kcg in range(KSHC // G):
                wt = wstream.tile([128, G, H], dt.bfloat16, tag="wih")
                nc.sync.dma_start(out=wt[:], in_=Wih[:, kcg * G:(kcg + 1) * G, :])
                for j in range(G):
                    kc = kcg * G + j
                    MM(ps_h0[:], lhsT=flatT_sb[:, kc, :], rhs=wt[:, j, :],
                       start=(kc == 0), stop=(kc == KSHC - 1))
            for kcg in range(KSHC // G):
                wt = wstream.tile([128, G, H], dt.bfloat16, tag="wih")
                nc.sync.dma_start(out=wt[:], in_=Wic[:, kcg * G:(kcg + 1) * G, :])
                for j in range(G):
                    kc = kcg * G + j
                    MM(ps_c0[:], lhsT=flatT_sb[:, kc, :], rhs=wt[:, j, :],
                       start=(kc == 0), stop=(kc == KSHC - 1))
            for kc in range(RSHC):
                MM(ps_C0[:], lhsT=relT_sb[:, kc, :], rhs=WiC_sb[:, kc, :],
                   start=(kc == 0), stop=(kc == RSHC - 1))

            # ---- att1^T = W_enc^T @ info^T + b_enc (resident, bf16) ----
            NJ = (BL * P) // 512
            for ma in range(2):
                for nj in range(NJ):
                    pt = ps.tile([128, 512], dt.float32, tag="bank", name="ps_att1")
                    for kd in range(2):
                        MM(pt[:], lhsT=Wenc_sb[:, kd, ma * 128:(ma + 1) * 128],
                           rhs=infoT_sb[:, kd, nj * 512:(nj + 1) * 512],
                           start=(kd == 0), stop=(kd == 1))
                    nc.scalar.activation(
                        out=att1T_sb[:, ma, nj * 512:(nj + 1) * 512], in_=pt[:],
                        func=AF.Identity, bias=benc_sb[:, ma:ma + 1])

            # ---- collective: ReduceScatter hands each core its 4 rows ----
            cc_sb = work.tile([32, 3, H], dt.float32, tag="cc_sb")
            nc.scalar.activation(out=cc_sb[:, 0, :], in_=ps_h0[:], func=AF.Copy)
            nc.scalar.activation(out=cc_sb[:, 1, :], in_=ps_c0[:], func=AF.Copy)
            nc.scalar.activation(out=cc_sb[:, 2, :], in_=ps_C0[:], func=AF.Copy)
            nc.sync.dma_start(out=cc_in[:], in_=cc_sb[:].rearrange("b s h -> b (s h)"))
            nc.gpsimd.collective_compute(
                "ReduceScatter", OP.add, replica_groups=[list(range(NC_))],
                ins=[cc_in[:]], outs=[cc_out[:]])
            st_sb = work.tile([BL, 3, 2, 128], dt.float32, tag="st_sb")
            nc.sync.dma_start(out=st_sb[:], in_=cc_out[:])

            # transpose [4,128] -> [128,4] and add init biases
            for s in range(3):
                for kc in range(2):
                    pt = ps.tile([128, BL], dt.float32, tag="bank", name="ps_tr")
                    nc.tensor.transpose(pt[:], st_sb[:, s, kc, :], id4_sb[:])
                    if s == 0:
                        nc.scalar.activation(out=hall_bf[:, kc, 0, :], in_=pt[:],
                                             func=AF.Identity,
                                             bias=bih_sb[:, kc:kc + 1])
                    elif s == 1:
                        nc.scalar.activation(out=cC[:, 0, kc, :], in_=pt[:],
                                             func=AF.Identity,
                                             bias=bic_sb[:, kc:kc + 1])
                    else:
                        nc.scalar.activation(out=cC[:, 1, kc, :], in_=pt[:],
                                             func=AF.Identity,
                                             bias=biC_sb[:, kc:kc + 1])

            # ---- recurrent loop ----
            for t in range(Tm):
                # att2^T = W_dec^T @ h^T + b_dec   -> bf16 [128, 2, 4]
                pa = ps.tile([128, 2, BL], dt.float32, tag="bank", name="ps_att2")
                first = True
                for ma in range(2):
                    for kh in range(2):
                        MM(pa[:, ma, :],
                           lhsT=Wdec_sb[:, kh, ma * 128:(ma + 1) * 128],
                           rhs=hall_bf[:, kh, t, :],
                           start=first, stop=(ma == 1 and kh == 1))
                        first = False
                att2_f = small.tile([128, 2, BL], dt.float32, tag="att2f")
                for ma in range(2):
                    nc.scalar.activation(out=att2_f[:, ma, :], in_=pa[:, ma, :],
                                         func=AF.Identity,
                                         bias=bdec_sb[:, ma:ma + 1])

                # R = relu(att1T + att2T) (bf16), e^T = R^T @ W_full  (psum)
                pe = ps.tile([128, BL, 8], dt.float32, tag="bank", name="ps_e")
                first = True
                for b in range(BL):
                    for ma in range(2):
                        rt = rpool.tile([128, P], dt.bfloat16, tag="R")
                        if (b * 2 + ma) in (0, 3, 6):
                            nc.scalar.activation(
                                out=rt[:], in_=att1T_sb[:, ma, b * P:(b + 1) * P],
                                func=AF.Relu, bias=att2_f[:, ma, b:b + 1])
                        else:
                            nc.vector.tensor_scalar(
                                out=rt[:], in0=att1T_sb[:, ma, b * P:(b + 1) * P],
                                scalar1=att2_f[:, ma, b:b + 1], scalar2=0.0,
                                op0=OP.add, op1=OP.max)
                        for po in range(8):
                            MM(pe[:, b, po:po + 1],
                               lhsT=rt[:, po * 128:(po + 1) * 128],
                               rhs=Wfull_sb[:, ma:ma + 1],
                               start=first, stop=(b == BL - 1 and ma == 1 and po == 7))
                            first = False

                # softmax (no max-shift; e is small by construction)
                exp_sb = small.tile([128, BL, 8], dt.float32, tag="exp")
                nc.scalar.activation(out=exp_sb[:], in_=pe[:], func=AF.Exp)
                psm = ps.tile([1, BL * 8], dt.float32, tag="bank", name="ps_sum")
                MM(psm[:], lhsT=onesa_sb[:], rhs=exp_sb[:])
                sums4 = small.tile([1, BL], dt.float32, tag="sums4")
                nc.vector.tensor_reduce(
                    out=sums4[:], in_=psm[:].rearrange("o (b q) -> o b q", q=8),
                    axis=mybir.AxisListType.X, op=OP.add)
                rcp4 = small.tile([1, BL], dt.float32, tag="rcp4")
                nc.vector.reciprocal(out=rcp4[:], in_=sums4[:])
                prb = ps.tile([128, BL], dt.float32, tag="bank", name="ps_rcpb")
                MM(prb[:], lhsT=onesb_sb[:], rhs=rcp4[:])

                alph_f = small.tile([128, BL, 8], dt.float32, tag="alphf")
                nc.vector.tensor_tensor(out=alph_f[:], in0=exp_sb[:],
                                        in1=prb[:].to_broadcast([128, BL, 8]),
                                        op=OP.mult)
                alph_bf = small.tile([128, BL, 8], dt.bfloat16, tag="alphbf")
                nc.vector.tensor_tensor(out=alph_bf[:], in0=exp_sb[:],
                                        in1=prb[:].to_broadcast([128, BL, 8]),
                                        op=OP.mult)
                nc.sync.dma_start(out=alph_o[t], in_=alph_f[:])

                # awf^T = info^T @ alpha  [128, 2, 4]
                paw = ps.tile([128, 2, BL], dt.float32, tag="bank", name="ps_awf")
                first = True
                for b in range(BL):
                    for ma in range(2):
                        for po in range(8):
                            MM(paw[:, ma, b:b + 1],
                               lhsT=infoc_sb[:, b * 8 + po, ma * 128:(ma + 1) * 128],
                               rhs=alph_bf[:, b, po:po + 1],
                               start=first,
                               stop=(b == BL - 1 and ma == 1 and po == 7))
                            first = False
                awf_bf = small.tile([128, 2, BL], dt.bfloat16, tag="awfbf")
                nc.vector.tensor_copy(out=awf_bf[:], in_=paw[:])

                # gates^T = W_gates^T @ [word; awf; h]
                sig_sb = small.tile([128, 3, 2, BL], dt.float32, tag="sig")
                g12_sb = small.tile([128, 2, 2, BL], dt.float32, tag="g12")
                for mg in range(10):
                    pg = ps.tile([128, BL], dt.float32, tag="bank", name="ps_g")
                    for kc in range(8):
                        if kc < 4:
                            rhs = wordsT_sb[:, kc, t, :]
                        elif kc < 6:
                            rhs = awf_bf[:, kc - 4, :]
                        else:
                            rhs = hall_bf[:, kc - 6, t, :]
                        MM(pg[:], lhsT=Wg_sb[:, kc, mg * 128:(mg + 1) * 128],
                           rhs=rhs, start=(kc == 0), stop=(kc == 7))
                    if mg < 6:
                        nc.scalar.activation(out=sig_sb[:, mg // 2, mg % 2, :],
                                             in_=pg[:], func=AF.Sigmoid,
                                             bias=bg_sb[:, mg:mg + 1])
                    else:
                        nc.scalar.activation(out=g12_sb[:, (mg - 6) // 2, (mg - 6) % 2, :],
                                             in_=pg[:], func=AF.Tanh,
                                             bias=bg_sb[:, mg:mg + 1])

                # state update: cC = f*cC + i*g12   (f,i broadcast over c/C)
                f4 = sig_sb[:, 1:2, :, :].to_broadcast([128, 2, 2, BL])
                i4 = sig_sb[:, 0:1, :, :].to_broadcast([128, 2, 2, BL])
                t1 = small.tile([128, 2, 2, BL], dt.float32, tag="t1")
                t2 = small.tile([128, 2, 2, BL], dt.float32, tag="t2")
                nc.vector.tensor_tensor(out=t1[:], in0=f4, in1=cC[:], op=OP.mult)
                nc.vector.tensor_tensor(out=t2[:], in0=i4, in1=g12_sb[:], op=OP.mult)
                nc.vector.tensor_tensor(out=cC[:], in0=t1[:], in1=t2[:], op=OP.add)
                cC_bf = small.tile([128, 2, 2, BL], dt.bfloat16, tag="cCbf")
                nc.vector.tensor_copy(out=cC_bf[:], in_=cC[:])

                # h = o * tanh(W_mlp^T @ [c;C] + b_mlp)  -> bf16 col t+1
                pm = ps.tile([128, 2, BL], dt.float32, tag="bank", name="ps_mlp")
                first = True
                for mh in range(2):
                    for kc in range(4):
                        MM(pm[:, mh, :],
                           lhsT=Wmlp_sb[:, kc, mh * 128:(mh + 1) * 128],
                           rhs=cC_bf[:, kc // 2, kc % 2, :],
                           start=first, stop=(mh == 1 and kc == 3))
                        first = False
                th = small.tile([128, 2, BL], dt.float32, tag="th")
                for mh in range(2):
                    nc.scalar.activation(out=th[:, mh, :], in_=pm[:, mh, :],
                                         func=AF.Tanh, bias=bmlp_sb[:, mh:mh + 1])
                nc.vector.tensor_tensor(out=hall_bf[:, :, t + 1, :],
                                        in0=sig_sb[:, 2, :, :], in1=th[:],
                                        op=OP.mult)

            # ---- predictions: one big matmul over all steps ----
            for nj in range(20):
                pf = ps.tile([128, 500], dt.float32, tag="bank", name="ps_fc")
                for kc in range(2):
                    MM(pf[:BL * Tm, :],
                       lhsT=hall_bf[:, kc, 1:T, :],
                       rhs=Wfc_sb[:, kc, nj * 500:(nj + 1) * 500],
                       start=(kc == 0), stop=(kc == 1))
                pred_sb = work.tile([BL * Tm, 500], dt.float32, tag="pred_sb")
                nc.scalar.activation(out=pred_sb[:], in_=pf[:BL * Tm, :],
                                     func=AF.Copy)
                nc.sync.dma_start(out=preds_o[:, nj * 500:(nj + 1) * 500],
                                  in_=pred_sb[:])

    nc.compile()
    return nc


def _get_nc():
    if "nc" not in _CACHE:
        _CACHE["nc"] = _build_nc()
    return _CACHE["nc"]


def prepare(inputs):
    """Host-side: sort, gather, shard, transpose, cast. Returns (in_maps, ctx)."""
    inp = {k: np.asarray(v) for k, v in inputs.items()}
    lens = np.asarray(inp["captions_lens"]).reshape(B)
    order = np.argsort(-lens, kind="stable")
    lens_s = lens[order]
    caps_s = np.asarray(inp["captions"]).reshape(B, T)[order]
    info_s = np.asarray(inp["info"], F32).reshape(B, P, D)[order]
    rel_s = np.asarray(inp["relation"], F32).reshape(B, REL)[order]
    sent_len = lens_s - 1
    mask = (sent_len[:, None] > np.arange(Tm)[None, :]).astype(F32)

    words = np.asarray(inp["emb"], F32)[caps_s]          # [32, 20, 512]
    flat = info_s.reshape(B, IN_FLAT)

    def chunkP(x):  # [K, N] -> [128, K//128, N]
        K, N = x.shape
        return np.ascontiguousarray(
            x.reshape(K // 128, 128, N).transpose(1, 0, 2))

    W_gates = np.concatenate([inp[f"W_{g}"] for g in ("i", "f", "o", "g1", "g2")],
                             axis=1).astype(F32)
    b_gates = np.concatenate([inp[f"b_{g}"] for g in ("i", "f", "o", "g1", "g2")]
                             ).astype(F32)

    shared = {
        "Wenc": chunkP(np.asarray(inp["W_enc"], F32)).astype(BF16),
        "Wdec": chunkP(np.asarray(inp["W_dec"], F32)).astype(BF16),
        "Wfull": np.ascontiguousarray(
            np.asarray(inp["W_full"], F32).reshape(2, 128).T).astype(BF16),
        "Wg": chunkP(W_gates).astype(BF16),
        "Wmlp": chunkP(np.asarray(inp["W_mlp"], F32)).astype(BF16),
        "Wfc": chunkP(np.asarray(inp["W_fc"], F32)).astype(BF16),
        "benc": np.ascontiguousarray(np.asarray(inp["b_enc"], F32).reshape(2, 128).T),
        "bdec": np.ascontiguousarray(np.asarray(inp["b_dec"], F32).reshape(2, 128).T),
        "bg": np.ascontiguousarray(b_gates.reshape(10, 128).T),
        "bmlp": np.ascontiguousarray(np.asarray(inp["b_mlp"], F32).reshape(2, 128).T),
        "bih": np.ascontiguousarray(np.asarray(inp["b_ih"], F32).reshape(2, 128).T),
        "bic": np.ascontiguousarray(np.asarray(inp["b_ic"], F32).reshape(2, 128).T),
        "biC": np.ascontiguousarray(np.asarray(inp["b_iC"], F32).reshape(2, 128).T),
        "ident4": np.eye(BL, dtype=F32),
        "ones_a": np.ones((128, 1), F32),
        "ones_b": np.ones((1, 128), F32),
    }

    W_ih = np.asarray(inp["W_ih"], F32)
    W_ic = np.asarray(inp["W_ic"], F32)
    W_iC = np.asarray(inp["W_iC"], F32)
    flatT_full = np.ascontiguousarray(flat.T)            # [262144, 32]
    relT_full = np.ascontiguousarray(rel_s.T)            # [16384, 32]

    in_maps = []
    for c in range(NC_):
        rows = slice(c * BL, (c + 1) * BL)
        ks = slice(c * KSH, (c + 1) * KSH)
        rs = slice(c * RSH, (c + 1) * RSH)
        il = info_s[rows]                                # [4, 1024, 256]
        m = dict(shared)
        m["flatT"] = chunkP(flatT_full[ks]).astype(BF16)
        m["Wih"] = chunkP(W_ih[ks]).astype(BF16)
        m["Wic"] = chunkP(W_ic[ks]).astype(BF16)
        m["relT"] = chunkP(relT_full[rs]).astype(BF16)
        m["WiC"] = chunkP(W_iC[rs]).astype(BF16)
        m["infoT"] = chunkP(
            np.ascontiguousarray(il.transpose(2, 0, 1).reshape(D, BL * P))
        ).astype(BF16)
        m["infoc"] = np.ascontiguousarray(
            il.reshape(BL, 8, 128, D).transpose(2, 0, 1, 3).reshape(128, BL * 8, D)
        ).astype(BF16)
        m["wordsT"] = np.ascontiguousarray(
            words[rows, :Tm, :].transpose(2, 1, 0).reshape(4, 128, Tm, BL)
            .transpose(1, 0, 2, 3)).astype(BF16)
        in_maps.append(m)

    ctx = {"order": order, "mask": mask, "b_fc": np.asarray(inp["b_fc"], F32)}
    return in_maps, ctx


def gather(results, ctx):
    mask, b_fc, order = ctx["mask"], ctx["b_fc"], ctx["order"]
    preds = np.zeros((B, Tm, V), F32)
    alphas = np.zeros((B, Tm, P), F32)
    for c in range(NC_):
        r = results[c]
        preds[c * BL:(c + 1) * BL] = r["preds_o"].reshape(Tm, BL, V).transpose(1, 0, 2)
        alphas[c * BL:(c + 1) * BL] = (
            r["alph_o"].transpose(2, 0, 3, 1).reshape(BL, Tm, P))
    preds = (preds + b_fc[None, None, :]) * mask[:, :, None]
    alphas = alphas * mask[:, :, None]
    return preds, alphas, order.astype(np.int32)


def kernel(**inputs):
    from concourse.bass_utils import run_bass_kernel_spmd
    in_maps, ctx = prepare(inputs)
    nc = _get_nc()
    res = run_bass_kernel_spmd(nc, in_maps, list(range(NC_)))
    return gather(res.results, ctx)


# revision 6
# speedup vs baseline: 1.0798x; 1.0798x over previous
"""Trainium2 Bass kernel for nn_Decoder (attention-LSTM caption decoder).

Strategy (8 NeuronCores):
  - Init matmuls (h0/c0 = flat @ W_ih/W_ic, C0 = rel @ W_iC) are K-sharded:
    each core reads 1/8 of the big weights (bf16), computes partial sums,
    and a ReduceScatter(add) hands core c the rows for its 4 batch elements.
  - The 19-step recurrent loop is batch-sharded (4 sequences per core) with
    all loop weights replicated in SBUF; no per-step collectives.
  - pred = h @ W_fc is hoisted out of the loop: h^T for all steps is stacked
    and one big matmul produces the predictions at the end.
  - Host (numpy) does: sort-by-length, embedding gather, layout transposes,
    bf16 casts, bias b_fc add + length masking of outputs.

All device tensors are laid out host-side as [128 partitions, ...] so every
DMA is a contiguous-line transfer.
"""

import numpy as np
import ml_dtypes

B, T = 32, 20
H, E, A, V = 256, 512, 256, 10000
P, D = 1024, 256
IN_FLAT = P * D          # 262144
REL = 128 * 128          # 16384
Tm = T - 1               # 19
NC_ = 8                  # cores
BL = B // NC_            # 4 sequences per core
KSH = IN_FLAT // NC_     # 32768 contraction rows per core
KSHC = KSH // 128        # 256 k-chunks per core
RSH = REL // NC_         # 2048
RSHC = RSH // 128        # 16

BF16 = ml_dtypes.bfloat16
F32 = np.float32

_CACHE = {}


def _build_nc():
    """Build + compile the SPMD Bass module (same NEFF for all 8 cores)."""
    import concourse.bacc as bacc
    import concourse.tile as tile
    from concourse import mybir

    dt = mybir.dt
    AF = mybir.ActivationFunctionType
    OP = mybir.AluOpType

    nc = bacc.Bacc("TRN2", target_bir_lowering=False, debug=False,
                   enable_asserts=False, num_devices=NC_)

    def din(name, shape, d=dt.bfloat16):
        return nc.dram_tensor(name, list(shape), d, kind="ExternalInput").ap()

    # per-core sharded inputs
    flatT = din("flatT", [128, KSHC, BL * NC_])          # rhs for init (N=32)
    Wih = din("Wih", [128, KSHC, H])
    Wic = din("Wic", [128, KSHC, H])
    relT = din("relT", [128, RSHC, BL * NC_])
    WiC = din("WiC", [128, RSHC, H])
    infoT = din("infoT", [128, 2, BL * P])               # [p, d-chunk, (b,pix)]
    infoc = din("infoc", [128, BL * 8, D])               # [pix, (b,po), d]
    wordsT = din("wordsT", [128, 4, Tm, BL])             # [p, e-chunk, t, b]
    # replicated weights
    Wenc = din("Wenc", [128, 2, A])
    Wdec = din("Wdec", [128, 2, A])
    Wfull = din("Wfull", [128, 2])
    Wg = din("Wg", [128, 8, 5 * H])
    Wmlp = din("Wmlp", [128, 4, H])
    Wfc = din("Wfc", [128, 2, V])
    benc = din("benc", [128, 2], dt.float32)
    bdec = din("bdec", [128, 2], dt.float32)
    bg = din("bg", [128, 10], dt.float32)
    bmlp = din("bmlp", [128, 2], dt.float32)
    bih = din("bih", [128, 2], dt.float32)
    bic = din("bic", [128, 2], dt.float32)
    biC = din("biC", [128, 2], dt.float32)
    ident4 = din("ident4", [BL, BL], dt.float32)
    ones_a = din("ones_a", [128, 1], dt.float32)
    ones_b = din("ones_b", [1, 128], dt.float32)

    preds_o = nc.dram_tensor("preds_o", [BL * Tm, V], dt.float32,
                             kind="ExternalOutput").ap()
    alph_o = nc.dram_tensor("alph_o", [Tm, 128, BL, 8], dt.float32,
                            kind="ExternalOutput").ap()

    cc_in = nc.dram_tensor("cc_in", [B, 3 * H], dt.float32).ap()
    cc_out = nc.dram_tensor("cc_out", [BL, 3, 2, 128], dt.float32).ap()

    with tile.TileContext(nc) as tc:
        from contextlib import ExitStack
        with ExitStack() as ctx:
            consts = ctx.enter_context(tc.tile_pool(name="consts", bufs=1))
            wstream = ctx.enter_context(tc.tile_pool(name="wstream", bufs=3))
            rpool = ctx.enter_context(tc.tile_pool(name="rpool", bufs=10))
            small = ctx.enter_context(tc.tile_pool(name="small", bufs=3))
            work = ctx.enter_context(tc.tile_pool(name="work", bufs=3))
            ps = ctx.enter_context(tc.tile_pool(name="ps", bufs=7, space="PSUM"))

            def cload(ap_dram, shape, d=dt.bfloat16, name=None):
                t = consts.tile(shape, d, tag=name)
                nc.sync.dma_start(out=t[:], in_=ap_dram)
                return t

            # stream-critical loads first so PE starts immediately
            flatT_sb = cload(flatT, [128, KSHC, 32], name="flatT")
            relT_sb = cload(relT, [128, RSHC, 32], name="relT")
            WiC_sb = cload(WiC, [128, RSHC, H], name="WiC")

            att1T_sb = consts.tile([128, 2, BL * P], dt.bfloat16, tag="att1T")
            hall_bf = consts.tile([128, 2, T, BL], dt.bfloat16, tag="hall")
            cC = consts.tile([128, 2, 2, BL], dt.float32, tag="cC")

            MM = nc.tensor.matmul

            # ---- init partial matmuls (K-sharded; out rows = global batch) ----
            ps_h0 = ps.tile([32, H], dt.float32, tag="bank", name="ps_h0")
            ps_c0 = ps.tile([32, H], dt.float32, tag="bank", name="ps_c0")
            ps_C0 = ps.tile([32, H], dt.float32, tag="bank", name="ps_C0")
            G = 8
            for kcg in range(KSHC // G):
                wt = wstream.tile([128, G, H], dt.bfloat16, tag="wih")
                nc.sync.dma_start(out=wt[:], in_=Wih[:, kcg * G:(kcg + 1) * G, :])
                for j in range(G):
                    kc = kcg * G + j
                    MM(ps_h0[:], lhsT=flatT_sb[:, kc, :], rhs=wt[:, j, :],
                       start=(kc == 0), stop=(kc == KSHC - 1))
            for kcg in range(KSHC // G):
                wt = wstream.tile([128, G, H], dt.bfloat16, tag="wih")
                nc.sync.dma_start(out=wt[:], in_=Wic[:, kcg * G:(kcg + 1) * G, :])
                for j in range(G):
                    kc = kcg * G + j
                    MM(ps_c0[:], lhsT=flatT_sb[:, kc, :], rhs=wt[:, j, :],
                       start=(kc == 0), stop=(kc == KSHC - 1))
            for kc in range(RSHC):
                MM(ps_C0[:], lhsT=relT_sb[:, kc, :], rhs=WiC_sb[:, kc, :],
                   start=(kc == 0), stop=(kc == RSHC - 1))

            # remaining resident loads (DMA-queued behind the big stream)
            infoT_sb = cload(infoT, [128, 2, BL * P], name="infoT")
            Wenc_sb = cload(Wenc, [128, 2, A], name="Wenc")
            benc_sb = cload(benc, [128, 2], dt.float32, "benc")
            infoc_sb = cload(infoc, [128, BL * 8, D], name="infoc")
            wordsT_sb = cload(wordsT, [128, 4, Tm, BL], name="wordsT")
            Wdec_sb = cload(Wdec, [128, 2, A], name="Wdec")
            Wfull_sb = cload(Wfull, [128, 2], name="Wfull")
            Wg_sb = cload(Wg, [128, 8, 5 * H], name="Wg")
            Wmlp_sb = cload(Wmlp, [128, 4, H], name="Wmlp")
            Wfc_sb = cload(Wfc, [128, 2, V], name="Wfc")
            bdec_sb = cload(bdec, [128, 2], dt.float32, "bdec")
            bg_sb = cload(bg, [128, 10], dt.float32, "bg")
            bmlp_sb = cload(bmlp, [128, 2], dt.float32, "bmlp")
            bih_sb = cload(bih, [128, 2], dt.float32, "bih")
            bic_sb = cload(bic, [128, 2], dt.float32, "bic")
            biC_sb = cload(biC, [128, 2], dt.float32, "biC")
            id4_sb = cload(ident4, [BL, BL], dt.float32, "id4")
            onesa_sb = cload(ones_a, [128, 1], dt.float32, "onesa")
            onesb_sb = cload(ones_b, [1, 128], dt.float32, "onesb")

            # ---- U = words @ W_gates[:512] + b_g, for all steps (t-major) ----
            U_sb = consts.tile([128, 10, Tm, BL], dt.float32, tag="U")
            for mg in range(10):
                pu = ps.tile([128, Tm * BL], dt.float32, tag="bank", name="ps_u")
                for kc in range(4):
                    MM(pu[:], lhsT=Wg_sb[:, kc, mg * 128:(mg + 1) * 128],
                       rhs=wordsT_sb[:, kc, :, :],
                       start=(kc == 0), stop=(kc == 3))
                nc.scalar.activation(out=U_sb[:, mg, :, :], in_=pu[:],
                                     func=AF.Identity, bias=bg_sb[:, mg:mg + 1])

            # ---- att1^T = W_enc^T @ info^T + b_enc (resident, bf16) ----
            NJ = (BL * P) // 512
            for ma in range(2):
                for nj in range(NJ):
                    pt = ps.tile([128, 512], dt.float32, tag="bank", name="ps_att1")
                    for kd in range(2):
                        MM(pt[:], lhsT=Wenc_sb[:, kd, ma * 128:(ma + 1) * 128],
                           rhs=infoT_sb[:, kd, nj * 512:(nj + 1) * 512],
                           start=(kd == 0), stop=(kd == 1))
                    nc.scalar.activation(
                        out=att1T_sb[:, ma, nj * 512:(nj + 1) * 512], in_=pt[:],
                        func=AF.Identity, bias=benc_sb[:, ma:ma + 1])

            # ---- collective: ReduceScatter hands each core its 4 rows ----
            cc_sb = work.tile([32, 3, H], dt.float32, tag="cc_sb")
            nc.scalar.activation(out=cc_sb[:, 0, :], in_=ps_h0[:], func=AF.Copy)
            nc.scalar.activation(out=cc_sb[:, 1, :], in_=ps_c0[:], func=AF.Copy)
            nc.scalar.activation(out=cc_sb[:, 2, :], in_=ps_C0[:], func=AF.Copy)
            nc.sync.dma_start(out=cc_in[:], in_=cc_sb[:].rearrange("b s h -> b (s h)"))
            nc.gpsimd.collective_compute(
                "ReduceScatter", OP.add, replica_groups=[list(range(NC_))],
                ins=[cc_in[:]], outs=[cc_out[:]])
            st_sb = work.tile([BL, 3, 2, 128], dt.float32, tag="st_sb")
            nc.sync.dma_start(out=st_sb[:], in_=cc_out[:])

            # transpose [4,128] -> [128,4] and add init biases
            for s in range(3):
                for kc in range(2):
                    pt = ps.tile([128, BL], dt.float32, tag="bank", name="ps_tr")
                    nc.tensor.transpose(pt[:], st_sb[:, s, kc, :], id4_sb[:])
                    if s == 0:
                        nc.scalar.activation(out=hall_bf[:, kc, 0, :], in_=pt[:],
                                             func=AF.Identity,
                                             bias=bih_sb[:, kc:kc + 1])
                    elif s == 1:
                        nc.scalar.activation(out=cC[:, 0, kc, :], in_=pt[:],
                                             func=AF.Identity,
                                             bias=bic_sb[:, kc:kc + 1])
                    else:
                        nc.scalar.activation(out=cC[:, 1, kc, :], in_=pt[:],
                                             func=AF.Identity,
                                             bias=biC_sb[:, kc:kc + 1])

            # ---- recurrent loop ----
            for t in range(Tm):
                # att2^T = W_dec^T @ h^T + b_dec   -> bf16 [128, 2, 4]
                pa = ps.tile([128, 2, BL], dt.float32, tag="bank", name="ps_att2")
                first = True
                for ma in range(2):
                    for kh in range(2):
                        MM(pa[:, ma, :],
                           lhsT=Wdec_sb[:, kh, ma * 128:(ma + 1) * 128],
                           rhs=hall_bf[:, kh, t, :],
                           start=first, stop=(ma == 1 and kh == 1))
                        first = False
                att2_f = small.tile([128, 2, BL], dt.float32, tag="att2f")
                for ma in range(2):
                    nc.vector.tensor_scalar(
                        out=att2_f[:, ma, :], in0=pa[:, ma, :],
                        scalar1=bdec_sb[:, ma:ma + 1], scalar2=None, op0=OP.add)

                # R = relu(att1T + att2T) (bf16), e^T = R^T @ W_full  (psum)
                pe = ps.tile([128, BL, 8], dt.float32, tag="bank", name="ps_e")
                first = True
                for b in range(BL):
                    for ma in range(2):
                        rt = rpool.tile([128, P], dt.bfloat16, tag="R")
                        if (b * 2 + ma) in (0, 3, 6):
                            nc.scalar.activation(
                                out=rt[:], in_=att1T_sb[:, ma, b * P:(b + 1) * P],
                                func=AF.Relu, bias=att2_f[:, ma, b:b + 1])
                        else:
                            nc.vector.tensor_scalar(
                                out=rt[:], in0=att1T_sb[:, ma, b * P:(b + 1) * P],
                                scalar1=att2_f[:, ma, b:b + 1], scalar2=0.0,
                                op0=OP.add, op1=OP.max)
                        for po in range(8):
                            MM(pe[:, b, po:po + 1],
                               lhsT=rt[:, po * 128:(po + 1) * 128],
                               rhs=Wfull_sb[:, ma:ma + 1],
                               start=first, stop=(b == BL - 1 and ma == 1 and po == 7))
                            first = False

                # softmax (no max-shift; e is small by construction)
                exp_sb = small.tile([128, BL, 8], dt.float32, tag="exp")
                nc.scalar.activation(out=exp_sb[:], in_=pe[:], func=AF.Exp)
                psm = ps.tile([1, BL * 8], dt.float32, tag="bank", name="ps_sum")
                MM(psm[:], lhsT=onesa_sb[:], rhs=exp_sb[:])
                sums4 = small.tile([1, BL], dt.float32, tag="sums4")
                nc.vector.tensor_reduce(
                    out=sums4[:], in_=psm[:].rearrange("o (b q) -> o b q", q=8),
                    axis=mybir.AxisListType.X, op=OP.add)
                rcp4 = small.tile([1, BL], dt.float32, tag="rcp4")
                nc.vector.reciprocal(out=rcp4[:], in_=sums4[:])
                prb = ps.tile([128, BL], dt.float32, tag="bank", name="ps_rcpb")
                MM(prb[:], lhsT=onesb_sb[:], rhs=rcp4[:])

                alph_f = small.tile([128, BL, 8], dt.float32, tag="alphf")
                nc.vector.tensor_tensor(out=alph_f[:], in0=exp_sb[:],
                                        in1=prb[:].to_broadcast([128, BL, 8]),
                                        op=OP.mult)
                alph_bf = small.tile([128, BL, 8], dt.bfloat16, tag="alphbf")
                nc.vector.tensor_tensor(out=alph_bf[:], in0=exp_sb[:],
                                        in1=prb[:].to_broadcast([128, BL, 8]),
                                        op=OP.mult)
                nc.sync.dma_start(out=alph_o[t], in_=alph_f[:])

                # awf^T = info^T @ alpha  [128, 2, 4]
                paw = ps.tile([128, 2, BL], dt.float32, tag="bank", name="ps_awf")
                first = True
                for b in range(BL):
                    for ma in range(2):
                        for po in range(8):
                            MM(paw[:, ma, b:b + 1],
                               lhsT=infoc_sb[:, b * 8 + po, ma * 128:(ma + 1) * 128],
                               rhs=alph_bf[:, b, po:po + 1],
                               start=first,
                               stop=(b == BL - 1 and ma == 1 and po == 7))
                            first = False
                awf_bf = small.tile([128, 2, BL], dt.bfloat16, tag="awfbf")
                nc.vector.tensor_copy(out=awf_bf[:], in_=paw[:])

                # gates^T = U_t + W_gates[512:]^T @ [awf; h]
                # sigmoid(x) computed as 0.5*tanh(x/2)+0.5 (stays in the
                # exp/tanh/relu ACT table set -- no table reloads)
                sig_sb = small.tile([128, 3, 2, BL], dt.float32, tag="sig")
                g12_sb = small.tile([128, 2, 2, BL], dt.float32, tag="g12")
                for mp in range(5):
                    pg = ps.tile([128, 2, BL], dt.float32, tag="bank", name="ps_g")
                    first = True
                    for mh in range(2):
                        mg = mp * 2 + mh
                        for kc in range(4, 8):
                            rhs = (awf_bf[:, kc - 4, :] if kc < 6
                                   else hall_bf[:, kc - 6, t, :])
                            MM(pg[:, mh, :],
                               lhsT=Wg_sb[:, kc, mg * 128:(mg + 1) * 128],
                               rhs=rhs, start=first, stop=(mh == 1 and kc == 7))
                            first = False
                    pre = small.tile([128, 2, BL], dt.float32, tag="pre")
                    nc.vector.tensor_tensor(out=pre[:], in0=pg[:],
                                            in1=U_sb[:, 2 * mp:2 * mp + 2, t, :],
                                            op=OP.add)
                    if mp < 3:
                        th_g = small.tile([128, 2, BL], dt.float32, tag="thg")
                        nc.scalar.activation(out=th_g[:], in_=pre[:],
                                             func=AF.Tanh, scale=0.5)
                        nc.vector.tensor_scalar(
                            out=sig_sb[:, mp, :, :], in0=th_g[:],
                            scalar1=0.5, scalar2=0.5, op0=OP.mult, op1=OP.add)
                    else:
                        nc.scalar.activation(out=g12_sb[:, mp - 3, :, :],
                                             in_=pre[:], func=AF.Tanh)

                # state update: cC = f*cC + i*g12   (f,i broadcast over c/C)
                f4 = sig_sb[:, 1:2, :, :].to_broadcast([128, 2, 2, BL])
                i4 = sig_sb[:, 0:1, :, :].to_broadcast([128, 2, 2, BL])
                t1 = small.tile([128, 2, 2, BL], dt.float32, tag="t1")
                t2 = small.tile([128, 2, 2, BL], dt.float32, tag="t2")
                nc.vector.tensor_tensor(out=t1[:], in0=f4, in1=cC[:], op=OP.mult)
                nc.vector.tensor_tensor(out=t2[:], in0=i4, in1=g12_sb[:], op=OP.mult)
                nc.vector.tensor_tensor(out=cC[:], in0=t1[:], in1=t2[:], op=OP.add)
                cC_bf = small.tile([128, 2, 2, BL], dt.bfloat16, tag="cCbf")
                nc.vector.tensor_copy(out=cC_bf[:], in_=cC[:])

                # h = o * tanh(W_mlp^T @ [c;C] + b_mlp)  -> bf16 col t+1
                pm = ps.tile([128, 2, BL], dt.float32, tag="bank", name="ps_mlp")
                first = True
                for mh in range(2):
                    for kc in range(4):
                        MM(pm[:, mh, :],
                           lhsT=Wmlp_sb[:, kc, mh * 128:(mh + 1) * 128],
                           rhs=cC_bf[:, kc // 2, kc % 2, :],
                           start=first, stop=(mh == 1 and kc == 3))
                        first = False
                pre_m = small.tile([128, 2, BL], dt.float32, tag="prem")
                nc.vector.tensor_tensor(
                    out=pre_m[:], in0=pm[:],
                    in1=bmlp_sb[:, :, None].to_broadcast([128, 2, BL]), op=OP.add)
                th = small.tile([128, 2, BL], dt.float32, tag="th")
                nc.scalar.activation(out=th[:], in_=pre_m[:], func=AF.Tanh)
                nc.vector.tensor_tensor(out=hall_bf[:, :, t + 1, :],
                                        in0=sig_sb[:, 2, :, :], in1=th[:],
                                        op=OP.mult)

            # ---- predictions: one big matmul over all steps ----
            for nj in range(20):
                pf = ps.tile([128, 500], dt.float32, tag="bank", name="ps_fc")
                for kc in range(2):
                    MM(pf[:BL * Tm, :],
                       lhsT=hall_bf[:, kc, 1:T, :],
                       rhs=Wfc_sb[:, kc, nj * 500:(nj + 1) * 500],
                       start=(kc == 0), stop=(kc == 1))
                pred_sb = work.tile([BL * Tm, 500], dt.float32, tag="pred_sb")
                if nj % 2 == 0:
                    nc.scalar.activation(out=pred_sb[:], in_=pf[:BL * Tm, :],
                                         func=AF.Copy)
                else:
                    nc.vector.tensor_copy(out=pred_sb[:], in_=pf[:BL * Tm, :])
                nc.sync.dma_start(out=preds_o[:, nj * 500:(nj + 1) * 500],
                                  in_=pred_sb[:])

    nc.compile()
    return nc


def _get_nc():
    if "nc" not in _CACHE:
        _CACHE["nc"] = _build_nc()
    return _CACHE["nc"]


def prepare(inputs):
    """Host-side: sort, gather, shard, transpose, cast. Returns (in_maps, ctx)."""
    inp = {k: np.asarray(v) for k, v in inputs.items()}
    lens = np.asarray(inp["captions_lens"]).reshape(B)
    order = np.argsort(-lens, kind="stable")
    lens_s = lens[order]
    caps_s = np.asarray(inp["captions"]).reshape(B, T)[order]
    info_s = np.asarray(inp["info"], F32).reshape(B, P, D)[order]
    rel_s = np.asarray(inp["relation"], F32).reshape(B, REL)[order]
    sent_len = lens_s - 1
    mask = (sent_len[:, None] > np.arange(Tm)[None, :]).astype(F32)

    words = np.asarray(inp["emb"], F32)[caps_s]          # [32, 20, 512]
    flat = info_s.reshape(B, IN_FLAT)

    def chunkP(x):  # [K, N] -> [128, K//128, N]
        K, N = x.shape
        return np.ascontiguousarray(
            x.reshape(K // 128, 128, N).transpose(1, 0, 2))

    W_gates = np.concatenate([inp[f"W_{g}"] for g in ("i", "f", "o", "g1", "g2")],
                             axis=1).astype(F32)
    b_gates = np.concatenate([inp[f"b_{g}"] for g in ("i", "f", "o", "g1", "g2")]
                             ).astype(F32)

    shared = {
        "Wenc": chunkP(np.asarray(inp["W_enc"], F32)).astype(BF16),
        "Wdec": chunkP(np.asarray(inp["W_dec"], F32)).astype(BF16),
        "Wfull": np.ascontiguousarray(
            np.asarray(inp["W_full"], F32).reshape(2, 128).T).astype(BF16),
        "Wg": chunkP(W_gates).astype(BF16),
        "Wmlp": chunkP(np.asarray(inp["W_mlp"], F32)).astype(BF16),
        "Wfc": chunkP(np.asarray(inp["W_fc"], F32)).astype(BF16),
        "benc": np.ascontiguousarray(np.asarray(inp["b_enc"], F32).reshape(2, 128).T),
        "bdec": np.ascontiguousarray(np.asarray(inp["b_dec"], F32).reshape(2, 128).T),
        "bg": np.ascontiguousarray(b_gates.reshape(10, 128).T),
        "bmlp": np.ascontiguousarray(np.asarray(inp["b_mlp"], F32).reshape(2, 128).T),
        "bih": np.ascontiguousarray(np.asarray(inp["b_ih"], F32).reshape(2, 128).T),
        "bic": np.ascontiguousarray(np.asarray(inp["b_ic"], F32).reshape(2, 128).T),
        "biC": np.ascontiguousarray(np.asarray(inp["b_iC"], F32).reshape(2, 128).T),
        "ident4": np.eye(BL, dtype=F32),
        "ones_a": np.ones((128, 1), F32),
        "ones_b": np.ones((1, 128), F32),
    }

    W_ih = np.asarray(inp["W_ih"], F32)
    W_ic = np.asarray(inp["W_ic"], F32)
    W_iC = np.asarray(inp["W_iC"], F32)
    flatT_full = np.ascontiguousarray(flat.T)            # [262144, 32]
    relT_full = np.ascontiguousarray(rel_s.T)            # [16384, 32]

    in_maps = []
    for c in range(NC_):
        rows = slice(c * BL, (c + 1) * BL)
        ks = slice(c * KSH, (c + 1) * KSH)
        rs = slice(c * RSH, (c + 1) * RSH)
        il = info_s[rows]                                # [4, 1024, 256]
        m = dict(shared)
        m["flatT"] = chunkP(flatT_full[ks]).astype(BF16)
        m["Wih"] = chunkP(W_ih[ks]).astype(BF16)
        m["Wic"] = chunkP(W_ic[ks]).astype(BF16)
        m["relT"] = chunkP(relT_full[rs]).astype(BF16)
        m["WiC"] = chunkP(W_iC[rs]).astype(BF16)
        m["infoT"] = chunkP(
            np.ascontiguousarray(il.transpose(2, 0, 1).reshape(D, BL * P))
        ).astype(BF16)
        m["infoc"] = np.ascontiguousarray(
            il.reshape(BL, 8, 128, D).transpose(2, 0, 1, 3).reshape(128, BL * 8, D)
        ).astype(BF16)
        m["wordsT"] = np.ascontiguousarray(
            words[rows, :Tm, :].transpose(2, 1, 0).reshape(4, 128, Tm, BL)
            .transpose(1, 0, 2, 3)).astype(BF16)
        in_maps.append(m)

    ctx = {"order": order, "mask": mask, "b_fc": np.asarray(inp["b_fc"], F32)}
    return in_maps, ctx


def gather(results, ctx):
    mask, b_fc, order = ctx["mask"], ctx["b_fc"], ctx["order"]
    preds = np.zeros((B, Tm, V), F32)
    alphas = np.zeros((B, Tm, P), F32)
    for c in range(NC_):
        r = results[c]
        preds[c * BL:(c + 1) * BL] = r["preds_o"].reshape(Tm, BL, V).transpose(1, 0, 2)
        alphas[c * BL:(c + 1) * BL] = (
            r["alph_o"].transpose(2, 0, 3, 1).reshape(BL, Tm, P))
    preds = (preds + b_fc[None, None, :]) * mask[:, :, None]
    alphas = alphas * mask[:, :, None]
    return preds, alphas, order.astype(np.int32)


def kernel(**inputs):
    from concourse.bass_utils import run_bass_kernel_spmd
    in_maps, ctx = prepare(inputs)
    nc = _get_nc()
    res = run_bass_kernel_spmd(nc, in_maps, list(range(NC_)))
    return gather(res.results, ctx)
